# revision 7
# baseline (speedup 1.0000x reference)
"""CornerPooling kernel for Trainium2 (Bass/Tile), batch-sharded over 8 NeuronCores.

Per core: n_img images [H, W], row-major in SBUF (H on partitions in 128-row
tiles, W on the free dim).  Stage outputs are bf16 (PE matmul rate is the same
as fp32r, but transposes are 1.5x faster and SBUF pressure halves); the input
x stays fp32r, PSUM stays fp32, and the final output is evicted as fp32.

Pipeline (BatchNorm folded to scalar affines on the host):
  x1   = relu(conv3x3(x, wa') + ca)
  i1   = suffix-max along W (reversed tensor_tensor_scan, free dim)
  i2   = suffix-max along H (PE transpose -> scan from PSUM -> transpose back)
  u    = i1 + i2            (fused into transpose-back PSUM eviction)
  s    = relu(conv3x3(u, wb') + kc*x + cbc)   (kc*x via DVE stt in place on PSUM)
  out1 = relu(conv3x3(s, wa') + ca)
  out  = we*relu(conv3x3(out1, wd) + bd) + be
         (folded into one Act eviction when we>0 and be==0)

Convs: per [128, W] full-width PSUM tile, per half-bank 3 accumulating matmuls
with a tridiagonal stationary operand (vertical taps) and +/-1-shifted moving
windows (horizontal taps, edge-truncated => SAME zero padding), plus a 4th
K=32 matmul adding cross-tile boundary-row contributions (one-hot patterns x
pre-rowconved boundary rows).  The boundary row-convs run on the GPSIMD engine
for bf16 stages (it is otherwise idle), on DVE for the fp32r first stage.
"""

import numpy as np

EPS = 1e-5
PT = 128  # partitions per tile


def _round_f32r(a: np.ndarray) -> np.ndarray:
    """Round fp32 array to float32r (round-to-nearest, low 12 mantissa bits zero)."""
    u = a.astype(np.float32).view(np.uint32)
    u = (u.astype(np.uint64) + 0x800) & 0xFFFFF000
    return u.astype(np.uint32).view(np.float32)


def _to_bf16(a: np.ndarray):
    import ml_dtypes

    return np.asarray(a, np.float32).astype(ml_dtypes.bfloat16)


def _tridiag(wcol: np.ndarray) -> np.ndarray:
    """T[k, p] = wcol[k - p + 1] for |k-p|<=1, shape [PT, PT]."""
    T = np.zeros((PT, PT), np.float32)
    for k in range(PT):
        for d in (-1, 0, 1):
            p = k - d
            if 0 <= p < PT:
                T[k, p] = wcol[d + 1]
    return T


def _host_consts(n_rt, stages_w, ca, cbc, bd, kc, we, be):
    """Build all DMA-able constant arrays from the folded conv weights.

    stages_w: list of 4 3x3 numpy arrays (wa', wb', wa', wd).
    Stage 0 runs in fp32r (moving operand is x); stages 1..3 in bf16.
    """
    tri32 = np.zeros((PT, 3 * PT), np.float32)
    for dj in range(3):
        tri32[:, dj * PT:(dj + 1) * PT] = _tridiag(stages_w[0][:, dj])
    tri16 = np.zeros((PT, 9 * PT), np.float32)
    for si in range(1, 4):
        w = stages_w[si]
        for dj in range(3):
            blk = (si - 1) * 3 + dj
            tri16[:, blk * PT:(blk + 1) * PT] = _tridiag(w[:, dj])
    # Boundary-fix one-hot patterns, per row-tile t: [32, PT] at free block t.
    #   partition t-1      -> top fix row for out-tile t  (adds to out row 0)
    #   partition 16 + t   -> bottom fix row for out-tile t (out row 127)
    pat = np.zeros((32, n_rt * PT), np.float32)
    for t in range(n_rt):
        if t >= 1:
            pat[t - 1, t * PT + 0] = 1.0
        if t <= n_rt - 2:
            pat[16 + t, t * PT + (PT - 1)] = 1.0
    # Row-conv taps for fix rows: partitions 0..15 (top fixes, sourced from the
    # row above) use weight row 0; 16..31 (bottom fixes) use row 2.
    taps = np.zeros((PT, 12), np.float32)
    for si, w in enumerate(stages_w):
        for j in range(3):
            taps[0:16, si * 3 + j] = w[0, j]
            taps[16:32, si * 3 + j] = w[2, j]
    sc = np.zeros((PT, 6), np.float32)
    sc[:, 0] = ca
    sc[:, 1] = cbc
    sc[:, 2] = bd
    sc[:, 3] = we
    sc[:, 4] = be
    sc[:, 5] = we * bd
    return {
        "tri32": _round_f32r(tri32),
        "tri16": _to_bf16(tri16),
        "pat32": _round_f32r(pat),
        "pat16": _to_bf16(pat),
        "taps": taps,
        "ident": _to_bf16(np.eye(PT, dtype=np.float32)),
        "sc": sc,
        "kc": np.full((PT, 1), kc, np.float32),
    }


def _build_nc(n_img: int, H: int, W: int, fuse_final: bool, num_devices: int = 8):
    """Build + compile the Bacc program for n_img images of [H, W] per core."""
    from contextlib import ExitStack

    import concourse.bacc as bacc
    import concourse.tile as tile
    import concourse.mybir as mybir

    f32 = mybir.dt.float32
    f32r = mybir.dt.float32r
    bf16 = mybir.dt.bfloat16
    Alu = mybir.AluOpType
    Act = mybir.ActivationFunctionType
    NEG = -3.0e38

    n_rt = H // PT          # row tiles per image
    n_ct = W // PT          # col tiles per image (for transposes)
    NH = W // 2             # matmul half width (one PSUM bank)
    WB = W + 2              # padded block width (zero col at each side)
    FW = max(n_rt * WB, n_ct * H)  # free width of one [PT, FW] image tensor

    nc = bacc.Bacc("TRN2", target_bir_lowering=False, debug=False,
                   num_devices=num_devices)
    x_d = nc.dram_tensor("x", [n_img, H, W], f32r, kind="ExternalInput").ap()
    t32_d = nc.dram_tensor("tri32", [PT, 3 * PT], f32r, kind="ExternalInput").ap()
    t16_d = nc.dram_tensor("tri16", [PT, 9 * PT], bf16, kind="ExternalInput").ap()
    p32_d = nc.dram_tensor("pat32", [32, n_rt * PT], f32r, kind="ExternalInput").ap()
    p16_d = nc.dram_tensor("pat16", [32, n_rt * PT], bf16, kind="ExternalInput").ap()
    taps_d = nc.dram_tensor("taps", [PT, 12], f32, kind="ExternalInput").ap()
    id_d = nc.dram_tensor("ident", [PT, PT], bf16, kind="ExternalInput").ap()
    sc_d = nc.dram_tensor("sc", [PT, 6], f32, kind="ExternalInput").ap()
    kc_d = nc.dram_tensor("kc", [PT, 1], f32, kind="ExternalInput").ap()
    out_d = nc.dram_tensor("out", [n_img, H, W], f32, kind="ExternalOutput").ap()

    with tile.TileContext(nc) as tc, ExitStack() as ctx:
        cpool = ctx.enter_context(tc.tile_pool(name="consts", bufs=1))
        big = ctx.enter_context(tc.tile_pool(name="big", bufs=1))
        fxp = ctx.enter_context(tc.tile_pool(name="fix", bufs=1))
        o2p = ctx.enter_context(tc.tile_pool(name="o2p", bufs=2))
        cps = ctx.enter_context(tc.tile_pool(name="cpsum", bufs=3, space="PSUM"))
        tps = ctx.enter_context(tc.tile_pool(name="tpsum", bufs=2, space="PSUM"))

        tri32 = cpool.tile([PT, 3 * PT], f32r)
        nc.sync.dma_start(tri32[:], t32_d[:])
        tri16 = cpool.tile([PT, 9 * PT], bf16)
        nc.sync.dma_start(tri16[:], t16_d[:])
        pat32 = cpool.tile([32, n_rt * PT], f32r)
        nc.sync.dma_start(pat32[:], p32_d[:])
        pat16 = cpool.tile([32, n_rt * PT], bf16)
        nc.sync.dma_start(pat16[:], p16_d[:])
        taps = cpool.tile([PT, 12], f32)
        nc.sync.dma_start(taps[:], taps_d[:])
        ident = cpool.tile([PT, PT], bf16)
        nc.sync.dma_start(ident[:], id_d[:])
        sc = cpool.tile([PT, 6], f32)
        nc.sync.dma_start(sc[:], sc_d[:])
        kc = cpool.tile([PT, 1], f32)
        nc.sync.dma_start(kc[:], kc_d[:])
        neg = cpool.tile([PT, 1], bf16)
        nc.vector.memset(neg[:], NEG)

        # persistent boundary-row staging (G: gathered rows, F: row-conved)
        G32 = fxp.tile([32, W + 2], f32r, tag="G32", name="G32")
        F32 = fxp.tile([32, W], f32r, tag="F32", name="F32")
        G16 = fxp.tile([32, W + 2], bf16, tag="G16", name="G16")
        F16 = fxp.tile([32, W], bf16, tag="F16", name="F16")
        # zero the SAME-padding edge columns once; gathers never write them
        nc.vector.memset(G32[:, 0:1].bitcast(f32), 0.0)
        nc.vector.memset(G32[:, W + 1:W + 2].bitcast(f32), 0.0)
        nc.vector.memset(G16[:, 0:1], 0.0)
        nc.vector.memset(G16[:, W + 1:W + 2], 0.0)

        X = [big.tile([PT, n_rt * WB], f32r, tag=t, name=t) for t in ("X0", "X1")]
        Bt = big.tile([PT, FW], bf16, tag="B", name="B")
        Ct = big.tile([PT, FW], bf16, tag="C", name="C")
        Dt = big.tile([PT, FW], bf16, tag="D", name="D")
        OUT = big.tile([PT, n_rt * W], f32, tag="O", name="O")

        def zero_edges(ti, cast=None):
            edges = ti[:, 0:n_rt * WB].rearrange(
                "p (t wb) -> p t wb", wb=WB)[:, :, 0:WB:W + 1]
            nc.vector.memset(edges if cast is None else edges.bitcast(cast), 0.0)

        for ti in (X[0], X[1]):
            zero_edges(ti, cast=f32)
        for ti in (Bt, Ct):
            zero_edges(ti)

        def emit_conv(src, si, dst, stt_x=None, final=False):
            """One conv stage over one image.

            dst = relu(conv3x3(src) + bias), with optional + kc*x (stt_x)
            or trailing we*(.)+be affine (final).
            """
            if si == 0:
                tri_blk = [tri32[:, dj * PT:(dj + 1) * PT] for dj in range(3)]
                pat, G, F, feng = pat32, G32, F32, nc.vector
            else:
                tri_blk = [tri16[:, ((si - 1) * 3 + dj) * PT:
                                  ((si - 1) * 3 + dj + 1) * PT]
                           for dj in range(3)]
                pat, G, F, feng = pat16, G16, F16, nc.vector
            # ---- boundary fix rows: gather + 3-tap row conv ----
            if n_rt > 1:
                last = src[PT - 1:PT, 0:n_rt * WB].rearrange(
                    "p (t wb) -> p t wb", wb=WB)
                first = src[0:1, 0:n_rt * WB].rearrange(
                    "p (t wb) -> p t wb", wb=WB)
                nc.sync.dma_start(G[0:n_rt - 1, 1:W + 1],
                                  last[:, 0:n_rt - 1, 1:W + 1])
                nc.sync.dma_start(G[16:16 + n_rt - 1, 1:W + 1],
                                  first[:, 1:n_rt, 1:W + 1])
            t0 = taps[0:32, si * 3:si * 3 + 1]
            t1 = taps[0:32, si * 3 + 1:si * 3 + 2]
            t2 = taps[0:32, si * 3 + 2:si * 3 + 3]
            if si == 0:
                feng.tensor_scalar(F[:], G[:, 2:W + 2].bitcast(f32), t2,
                                   None, Alu.mult)
                feng.scalar_tensor_tensor(F[:], G[:, 1:W + 1].bitcast(f32),
                                          t1, F[:].bitcast(f32),
                                          Alu.mult, Alu.add)
                feng.scalar_tensor_tensor(F[:], G[:, 0:W].bitcast(f32),
                                          t0, F[:].bitcast(f32),
                                          Alu.mult, Alu.add)
            else:
                feng.tensor_scalar(F[:], G[:, 2:W + 2], t2, None, Alu.mult)
                feng.scalar_tensor_tensor(F[:], G[:, 1:W + 1], t1, F[:],
                                          Alu.mult, Alu.add)
                feng.scalar_tensor_tensor(F[:], G[:, 0:W], t0, F[:],
                                          Alu.mult, Alu.add)

            # ---- per tile: 2x(3 tridiag MMs + fix MM) -> one eviction ----
            for t in range(n_rt):
                ps = cps.tile([PT, W], f32, tag="cps", name="cps")
                for h in range(2):
                    c0 = h * NH
                    for dj in range(3):
                        nc.tensor.matmul(
                            ps[:, c0:c0 + NH],
                            tri_blk[dj],
                            src[:, t * WB + c0 + dj:t * WB + c0 + dj + NH],
                            start=(dj == 0), stop=False)
                    nc.tensor.matmul(
                        ps[:, c0:c0 + NH],
                        pat[:, t * PT:(t + 1) * PT],
                        F[:, c0:c0 + NH],
                        start=False, stop=True,
                        tile_position=(0, 0))
                d0 = t * WB + 1
                if stt_x is not None:
                    # ps += kc * x, in place, then relu(ps + cbc) -> dst
                    nc.vector.scalar_tensor_tensor(
                        ps[:], stt_x[:, d0:d0 + W].bitcast(f32),
                        kc[:], ps[:], Alu.mult, Alu.add)
                    nc.scalar.activation(dst[:, d0:d0 + W], ps[:],
                                         Act.Relu, bias=sc[:, 1:2])
                elif final:
                    if fuse_final:
                        # out = relu(we*ps + we*bd)  (we>0, be==0)
                        nc.scalar.activation(OUT[:, t * W:(t + 1) * W], ps[:],
                                             Act.Relu, bias=sc[:, 5:6],
                                             scale=sc[:, 3:4])
                    else:
                        o2 = o2p.tile([PT, W], bf16, tag="o2", name="o2")
                        nc.scalar.activation(o2[:], ps[:], Act.Relu,
                                             bias=sc[:, 2:3])
                        nc.vector.tensor_scalar(
                            OUT[:, t * W:(t + 1) * W], o2[:],
                            sc[:, 3:4], sc[:, 4:5], Alu.mult, Alu.add)
                else:
                    bias = sc[:, 0:1]
                    nc.scalar.activation(dst[:, d0:d0 + W], ps[:],
                                         Act.Relu, bias=bias)

        # ================= pipeline =================
        for i in range(n_img):
            xb = X[i % 2]
            for t in range(n_rt):
                nc.sync.dma_start(xb[:, t * WB + 1:t * WB + 1 + W],
                                  x_d[i, t * PT:(t + 1) * PT, :])

            # S1: x1 = relu(conv_a(x) + ca)
            emit_conv(xb, 0, Bt)

            # S2a: i1 = W-suffix-max of x1
            for t in range(n_rt):
                w0 = t * WB + 1
                nc.vector.tensor_tensor_scan(
                    Ct[:, w0:w0 + W][:, ::-1],
                    Bt[:, w0:w0 + W][:, ::-1],
                    Bt[:, w0:w0 + W][:, ::-1],
                    NEG, Alu.max, Alu.max)

            # S2b: transpose x1 -> PSUM; i2T = H-suffix-max straight off PSUM
            for c in range(n_ct):
                tp = tps.tile([PT, H], bf16, tag="tp", name="tp")
                for r in range(n_rt):
                    nc.tensor.transpose(
                        tp[:, r * PT:(r + 1) * PT],
                        Bt[:, r * WB + 1 + c * PT:r * WB + 1 + (c + 1) * PT],
                        ident[:])
                nc.vector.tensor_tensor_scan(
                    Dt[:, c * H:(c + 1) * H][:, ::-1],
                    tp[:][:, ::-1],
                    neg[:].to_broadcast((PT, H)),
                    NEG, Alu.max, Alu.max)

            # S2c: transpose i2T back; u = psum + i1 in place in C
            for r in range(n_rt):
                tp = tps.tile([PT, W], bf16, tag="tp", name="tp")
                for c in range(n_ct):
                    nc.tensor.transpose(
                        tp[:, c * PT:(c + 1) * PT],
                        Dt[:, c * H + r * PT:c * H + (r + 1) * PT],
                        ident[:])
                nc.vector.tensor_add(
                    Ct[:, r * WB + 1:r * WB + 1 + W],
                    tp[:],
                    Ct[:, r * WB + 1:r * WB + 1 + W])

            # S3: s = relu(conv_b(u) + kc*x + cbc)
            emit_conv(Ct, 1, Bt, stt_x=xb)

            # D was written with scan geometry in S2b; restore its zero edges
            # before it becomes conv input/output again.
            zero_edges(Dt)

            # S4: out1 = relu(conv_a(s) + ca)
            emit_conv(Bt, 2, Dt)

            # S5+S6: out = we*relu(conv_d(out1) + bd) + be -> OUT (f32) -> DMA
            emit_conv(Dt, 3, None, final=True)

            for t in range(n_rt):
                nc.sync.dma_start(out_d[i, t * PT:(t + 1) * PT, :],
                                  OUT[:, t * W:(t + 1) * W])

    nc.compile()
    return nc


def _fold_consts(wa, ba, ga, bta, ma, va, wb, bb, gb, btb, mb, vb,
                 wc, bc, gc, btc, mc, vc, wd, bd, we, be):
    sa = float(ga[0]) / np.sqrt(float(va[0]) + EPS)
    wa_e = (wa[0, 0].astype(np.float64) * sa).astype(np.float32)
    ca = (float(ba[0]) - float(ma[0])) * sa + float(bta[0])
    sb = float(gb[0]) / np.sqrt(float(vb[0]) + EPS)
    wb_e = (wb[0, 0].astype(np.float64) * sb).astype(np.float32)
    cb = (float(bb[0]) - float(mb[0])) * sb + float(btb[0])
    s_c = float(gc[0]) / np.sqrt(float(vc[0]) + EPS)
    kc = float(wc[0, 0, 0, 0]) * s_c
    cc = (float(bc[0]) - float(mc[0])) * s_c + float(btc[0])
    return (wa_e, ca, wb_e, cb + cc, wd[0, 0].astype(np.float32),
            float(bd[0]), kc, float(we[0, 0, 0, 0]), float(be[0]))


_NC_CACHE = {}


def _get_nc(n_img, H, W, fuse_final):
    key = (n_img, H, W, fuse_final)
    if key not in _NC_CACHE:
        _NC_CACHE[key] = _build_nc(n_img, H, W, fuse_final)
    return _NC_CACHE[key]


def _prepare(inputs):
    x = np.asarray(inputs["x"], np.float32)
    B, _, H, W = x.shape  # 32, 1, 1024, 1024
    n_cores = 8
    n_img = B // n_cores

    (wa_e, ca, wb_e, cbc, wd_e, bd_c, kc_c, we_c, be_c) = _fold_consts(
        *[np.asarray(inputs[k]) for k in
          ("wa", "ba", "ga", "bta", "ma", "va", "wb", "bb", "gb", "btb",
           "mb", "vb", "wc", "bc", "gc", "btc", "mc", "vc", "wd", "bd",
           "we", "be")])

    fuse_final = (we_c > 0.0) and (be_c == 0.0)
    consts = _host_consts(H // PT, [wa_e, wb_e, wa_e, wd_e],
                          ca, cbc, bd_c, kc_c, we_c, be_c)
    nc = _get_nc(n_img, H, W, fuse_final)

    xr = _round_f32r(x[:, 0])
    in_maps = []
    for c in range(n_cores):
        in_maps.append({
            "x": np.ascontiguousarray(xr[c * n_img:(c + 1) * n_img]),
            "tri32": consts["tri32"], "tri16": consts["tri16"],
            "pat32": consts["pat32"], "pat16": consts["pat16"],
            "taps": consts["taps"], "ident": consts["ident"],
            "sc": consts["sc"], "kc": consts["kc"]})
    return nc, in_maps, (B, n_img, H, W)


def kernel(**inputs) -> np.ndarray:
    from concourse.bass_utils import run_bass_kernel_spmd

    nc, in_maps, (B, n_img, H, W) = _prepare(inputs)
    res = run_bass_kernel_spmd(nc, in_maps, core_ids=list(range(len(in_maps))))
    out = np.empty((B, 1, H, W), np.float32)
    for c in range(len(in_maps)):
        out[c * n_img:(c + 1) * n_img, 0] = res.results[c]["out"]
    return out


def timed_run(inputs):
    """Run with NTFF tracing; return max exec_time_ns across traced cores."""
    from concourse.bass_utils import run_bass_kernel_spmd

    nc, in_maps, _ = _prepare(inputs)
    res = run_bass_kernel_spmd(nc, in_maps, core_ids=list(range(len(in_maps))),
                               trace=True)
    return res.exec_time_ns
